# revision 14
# baseline (speedup 1.0000x reference)
# Causal gated D2 (linear) attention — Trainium2 Bass kernel, 8 NeuronCores.
#
# Sharding: core c -> batch b = c // 4, head group g = c % 4 (heads 3g..3g+2).
# Each core computes LN-stats, qkv/gate projections, chunked causal linear
# attention for its 3 heads, and a partial output projection (its 192
# attention dims x full D). Host sums the 4 partial proj outputs per batch.
#
# v2 design notes (vs the 110us baseline):
#  - LN-stats matmuls run in fp8e4 DoubleRow (2x contraction, 0.5 cyc/row):
#    host uploads x and x^2 pre-quantized to e4m3. Projections stay bf16
#    (fp8 projections measured 3.5e-2 rel err -- too lossy).
#  - All DMA transposes replaced by PE-array transposes (matmul
#    is_transpose=True with an identity rhs) + PSUM->SBUF copies.
#  - Activation-table discipline: per lc-half the scalar engine runs
#    [sigmoid]*2 then [exp]*4 batches; stats use the shared ln+exp table.
#    Phase-2 reciprocals moved to the vector engine (reciprocal_approx_fast),
#    so phase 2 loads no tables at all.
#  - Output projection computed feature-major (out = W_proj^T @ attn^T,
#    moving dim 512), bf16 partials; host accumulates in f32.
#  - Emission order interleaves: stats -> [z(lc) -> elu(lc) -> transposes ->
#    attn chunks of lc -> proj-out(lc)] for lc in 0,1 so TensorMatrix stays
#    continuously busy while Vector/Scalar trail one stage behind.
#
# Feature packing (6 tiles of 128 partitions; per-head partition bases must
# match between q/k (matmul lhsT/rhs) and k/gate (elementwise)):
#   t0 = [g0 | g1]   t1 = [g2 | v0]   t2 = [k0 | k1]
#   t3 = [k2 | v2]   t4 = [q0 | q1]   t5 = [q2 | v1]
# Per 128-token block the PE transposes t2 ([k0n|k1n]), t3 ([k2n|v2n]),
# t1 (v0n half), t5 (v1n half) feed a nat tile
#   [v0n | 1 | v1n | 1 | v2n | 1 | k0n k1n k2n]  (cols 0..576)
# whose [v_h | ones] pairs are contiguous 128-col matmul operands.

import numpy as np

B, L, D, H = 2, 1024, 768, 12
DH = 64
NCORES = 8
HPC = 3          # heads per core
GROUPS = 4       # head groups per batch
CHUNK = 256      # attention chunk length
NCH = L // CHUNK

_compiled = {}


def _split_drain_tile_context():
    """TileContext that caps sem waits per instruction (this walrus build
    rejects instructions carrying more than one sync wait on several
    instruction templates) by moving excess waits onto preceding
    same-engine nops, and splits the tail drain the same way."""
    import concourse.tile as tile
    import concourse.mybir as mybir
    from concourse.tile import ScopedClock
    import bass_rust

    MAXW = 1

    class SplitDrainTileContext(tile.TileContext):
        _wsplit_counter = 0

        def _lower_ordered_insts(self, ordered):
            for bb_name, insts in ordered.items():
                need = any(
                    getattr(i, "sync_info", None) is not None
                    and len(i.sync_info.on_wait) > MAXW
                    for i in insts
                )
                if not need:
                    continue
                new_list = []
                for inst in insts:
                    si = getattr(inst, "sync_info", None)
                    if si is not None and len(si.on_wait) > MAXW:
                        waits = list(si.on_wait)
                        si.on_wait.clear()
                        for w in waits[MAXW:]:
                            SplitDrainTileContext._wsplit_counter += 1
                            nop = mybir.InstNoOp(
                                name=f"__wsplit_{SplitDrainTileContext._wsplit_counter}",
                                ins=[], outs=[],
                            )
                            nop.engine = inst.engine
                            nop.sync_info = bass_rust.SyncInfo(
                                on_wait=[w], on_update=[]
                            )
                            new_list.append(nop)
                        for w in waits[:MAXW]:
                            si.on_wait.append(w)
                    new_list.append(inst)
                insts[:] = new_list
            return super()._lower_ordered_insts(ordered)

        def _drain_and_barrier(self, tick_clock, wait_clock):
            drain_inst = self.nc.sync.drain()
            wait_clock.add_sem_waits(
                drain_inst.ins, ScopedClock({None: tick_clock.global_clock})
            )
            si = drain_inst.ins.sync_info
            if si is not None and len(si.on_wait) > 1:
                waits = list(si.on_wait)
                si.on_wait.clear()
                si.on_wait.append(waits[0])
                for w in waits[1:]:
                    d2 = self.nc.sync.drain()
                    si2 = d2.ins.sync_info
                    if si2 is None:
                        d2.ins.sync_info = bass_rust.SyncInfo(
                            on_wait=[w], on_update=[]
                        )
                    else:
                        si2.on_wait.append(w)
            self.nc.all_engine_barrier()
            assert self.sems is not None
            popped = self.nc._tile_sem_poison_stack.pop()
            assert popped is self._sem_poison
            self.nc.clear_and_free_semaphores(list(self.sems.allocated().values()))
            self.nc.all_engine_barrier()

    return SplitDrainTileContext


# (tile, partition base) per head; q/k/g share a base per head.
G_POS = [(0, 0), (0, 64), (1, 0)]
V_POS = [(1, 64), (5, 64), (3, 64)]
K_POS = [(2, 0), (2, 64), (3, 0)]
Q_POS = [(4, 0), (4, 64), (5, 0)]

# nat tile columns
V_COL = [0, 128, 256]          # start of [v_h | ones] 128-col run
K_COL = [384, 448, 512]        # start of k_h nat 64-col run


def _build_nc(has_qkv_bias, has_gate_bias):
    import concourse.bass as bass
    import concourse.mybir as mybir

    f32 = mybir.dt.float32
    bf16 = mybir.dt.bfloat16
    fp8 = mybir.dt.float8e4
    Alu = mybir.AluOpType
    Act = mybir.ActivationFunctionType
    DR = mybir.MatmulPerfMode.DoubleRow

    TC = _split_drain_tile_context()

    nc = bass.Bass()
    # ---- DRAM I/O ----
    xT = nc.dram_tensor("xT", (D, L), bf16, kind="ExternalInput")
    x8T = nc.dram_tensor("x8T", (D, L), fp8, kind="ExternalInput")
    x28T = nc.dram_tensor("x28T", (D, L), fp8, kind="ExternalInput")
    wAll = nc.dram_tensor("wAll", (D, 768), bf16, kind="ExternalInput")
    ones8 = nc.dram_tensor("ones8", (128, 256), fp8, kind="ExternalInput")
    idI = nc.dram_tensor("idI", (128, 128), bf16, kind="ExternalInput")
    maskI = nc.dram_tensor("maskI", (128, 512), bf16, kind="ExternalInput")
    negcs = nc.dram_tensor("negcs", (6, 128), f32, kind="ExternalInput")
    wp01 = nc.dram_tensor("wp01", (128, D), bf16, kind="ExternalInput")
    wp2 = nc.dram_tensor("wp2", (64, D), bf16, kind="ExternalInput")
    if has_qkv_bias:
        qkvbI = nc.dram_tensor("qkvbI", (6, 128), f32, kind="ExternalInput")
    if has_gate_bias:
        gbI = nc.dram_tensor("gbI", (6, 128), f32, kind="ExternalInput")
    outPT = nc.dram_tensor("outPT", (D, L), bf16, kind="ExternalOutput")
    gateT = nc.dram_tensor("gateT", (192, L), bf16, kind="ExternalOutput")

    with TC(nc) as tc:
        with tc.tile_pool(name="const", bufs=1) as cp:
            # ---- persistent SBUF ----
            x8_sb = cp.tile([128, 6, L], fp8)
            x28_sb = cp.tile([128, 6, L], fp8)
            xT_sb = cp.tile([128, 6, L], bf16)
            wAll_sb = cp.tile([128, 6, 768], bf16)
            ones8_sb = cp.tile([128, 2, 128], fp8)
            id_sb = cp.tile([128, 128], bf16)
            mask_sb = cp.tile([128, 512], bf16)
            negcs_sb = cp.tile([128, 6], f32)
            wp01_sb = cp.tile([128, 768], bf16)
            wp2_sb = cp.tile([64, 768], bf16)

            x8r = x8T.rearrange("(ks p) l -> p ks l", p=128)
            x28r = x28T.rearrange("(ks p) l -> p ks l", p=128)
            xTr = xT.rearrange("(ks p) l -> p ks l", p=128)
            wAr = wAll.rearrange("(ks p) m -> p ks m", p=128)
            outPr = outPT.rearrange("(ks p) l -> p ks l", p=128)

            # input DMAs, spread across queues, earliest-needed first
            nc.sync.dma_start(x8_sb[:, :, 0:512], x8r[:, :, 0:512])
            nc.sync.dma_start(x28_sb[:, :, 0:512], x28r[:, :, 0:512])
            nc.scalar.dma_start(x8_sb[:, :, 512:1024], x8r[:, :, 512:1024])
            nc.scalar.dma_start(x28_sb[:, :, 512:1024], x28r[:, :, 512:1024])
            nc.scalar.dma_start(wAll_sb[:, :, 0:384], wAr[:, :, 0:384])
            nc.scalar.dma_start(wAll_sb[:, :, 384:768], wAr[:, :, 384:768])
            nc.gpsimd.dma_start(xT_sb[:, :, 0:512], xTr[:, :, 0:512])
            nc.gpsimd.dma_start(xT_sb[:, :, 512:1024], xTr[:, :, 512:1024])
            nc.gpsimd.dma_start(wp01_sb[:], wp01[:])
            nc.gpsimd.dma_start(wp2_sb[:], wp2[:])
            nc.sync.dma_start(ones8_sb[:], ones8.rearrange("p (t m) -> p t m", t=2))
            nc.sync.dma_start(id_sb[:], idI[:])
            nc.sync.dma_start(mask_sb[:], maskI[:])
            nc.sync.dma_start(negcs_sb[:], negcs.rearrange("m p -> p m"))
            if has_qkv_bias:
                qkvb_sb = cp.tile([128, 6], f32)
                nc.sync.dma_start(qkvb_sb[:], qkvbI.rearrange("m p -> p m"))
            if has_gate_bias:
                gb_sb = cp.tile([128, 6], f32)
                nc.sync.dma_start(gb_sb[:], gbI.rearrange("m p -> p m"))

            epsb = cp.tile([128, 1], f32)
            nc.vector.memset(epsb[:], 1e-5)

            QK = cp.tile([128, 6, L], bf16)       # corrected q/k/v store
            gf = cp.tile([128, L], bf16)          # gate [g0|g1]
            gf2 = cp.tile([64, L], bf16)          # gate g2
            rstdB = cp.tile([128, L], f32)
            muB = cp.tile([128, L], f32)
            attnT01 = cp.tile([128, L], bf16)     # attention out heads 0,1
            attnT2 = cp.tile([64, L], bf16)       # attention out head 2

            # nat tiles, one per chunk: [128 tok, 2 jb, 576]
            nats = [cp.tile([128, 2, 576], bf16, name=f"nat{c}")
                    for c in range(NCH)]
            for c in range(NCH):
                for h in range(HPC):
                    eng = nc.vector if (c + h) % 2 else nc.gpsimd
                    eng.memset(nats[c][:, :, V_COL[h] + 64:V_COL[h] + 128], 1.0)

            # ---------- LN stats (fp8 DoubleRow) ----------
            with tc.tile_pool(name="stps", bufs=4, space="PSUM") as stps, \
                 tc.tile_pool(name="stwk", bufs=4) as stwk:
                st_ps = []
                for lc in range(2):
                    sl = slice(512 * lc, 512 * lc + 512)
                    sum_ps = stps.tile([128, 512], f32, tag="st",
                                       name=f"sum{lc}")
                    ssq_ps = stps.tile([128, 512], f32, tag="st",
                                       name=f"ssq{lc}")
                    for t in range(3):
                        nc.tensor.matmul(
                            sum_ps[:], lhsT=ones8_sb[:],
                            rhs=x8_sb[:, 2 * t:2 * t + 2, sl],
                            start=(t == 0), stop=(t == 2), perf_mode=DR,
                        )
                    for t in range(3):
                        nc.tensor.matmul(
                            ssq_ps[:], lhsT=ones8_sb[:],
                            rhs=x28_sb[:, 2 * t:2 * t + 2, sl],
                            start=(t == 0), stop=(t == 2), perf_mode=DR,
                        )
                    st_ps.append((sum_ps, ssq_ps))
                for lc in range(2):
                    sl = slice(512 * lc, 512 * lc + 512)
                    sum_ps, ssq_ps = st_ps[lc]
                    nc.scalar.activation(
                        out=muB[:, sl], in_=sum_ps[:], func=Act.Copy,
                        scale=1.0 / D,
                    )
                    msq = stwk.tile([128, 512], f32, tag="msq")
                    nc.scalar.activation(
                        out=msq[:], in_=muB[:, sl], func=Act.Square,
                    )
                    var = stwk.tile([128, 512], f32, tag="var")
                    nc.vector.scalar_tensor_tensor(
                        var[:], in0=ssq_ps[:], scalar=1.0 / D, in1=msq[:],
                        op0=Alu.mult, op1=Alu.subtract,
                    )
                    # rstd = exp(-0.5 * ln(var + eps))
                    nc.scalar.activation(
                        out=var[:], in_=var[:], func=Act.Ln, bias=epsb[:, 0:1]
                    )
                    nc.scalar.activation(
                        out=rstdB[:, sl], in_=var[:], func=Act.Exp, scale=-0.5
                    )

            # ---------- projections + attention, interleaved per lc ----------
            with tc.tile_pool(name="zps", bufs=3, space="PSUM") as zps, \
                 tc.tile_pool(name="tps", bufs=1, space="PSUM") as tps, \
                 tc.tile_pool(name="aps", bufs=2, space="PSUM") as aps, \
                 tc.tile_pool(name="ops", bufs=1, space="PSUM") as ops, \
                 tc.tile_pool(name="sps", bufs=1, space="PSUM") as sps, \
                 tc.tile_pool(name="wk", bufs=3) as wk, \
                 tc.tile_pool(name="tmn", bufs=8) as tmn, \
                 tc.tile_pool(name="asb", bufs=3) as asb, \
                 tc.tile_pool(name="rcb", bufs=3) as rcb, \
                 tc.tile_pool(name="ssb", bufs=6) as ssb, \
                 tc.tile_pool(name="posb", bufs=2) as posb:

                # persistent PSUM tiles with manual region alternation
                # (per-buffer PSUM allocation is bank-granular, so pools of
                # small tiles waste banks; regions of one tile still get
                # region-level dependency tracking)
                t_all = tps.tile([128, 8, 128], bf16, name="t_all")  # 1 bank
                o_all = ops.tile([128, 512], f32, name="o_all")      # 1 bank
                s_all = sps.tile([64, 2, 128], f32, name="s_all")    # 1 bank

                def mm6(m, sl):
                    z = zps.tile([128, 512], f32, tag="z")
                    for ks in range(6):
                        nc.tensor.matmul(
                            z[:], lhsT=wAll_sb[:, ks, 128 * m:128 * m + 128],
                            rhs=xT_sb[:, ks, sl],
                            start=(ks == 0), stop=(ks == 5),
                        )
                    return z

                def corr(eng, z, m, rows, sl):
                    # QK[rows, m, sl] = (z + mu*negcs) * rstd  (+ bias)
                    tq = wk.tile([128, 512], f32, tag="tq")
                    eng.scalar_tensor_tensor(
                        tq[rows], in0=muB[rows, sl],
                        scalar=negcs_sb[rows, m:m + 1],
                        in1=z[rows], op0=Alu.mult, op1=Alu.add,
                    )
                    if has_qkv_bias:
                        t2q = wk.tile([128, 512], f32, tag="t2q")
                        eng.tensor_tensor(
                            t2q[rows], tq[rows], rstdB[rows, sl], op=Alu.mult
                        )
                        eng.tensor_scalar_add(
                            out=QK[rows, m, sl], in0=t2q[rows],
                            scalar1=qkvb_sb[rows, m:m + 1],
                        )
                    else:
                        eng.tensor_tensor(
                            QK[rows, m, sl], tq[rows], rstdB[rows, sl],
                            op=Alu.mult,
                        )

                S_prev = [None] * HPC

                def attn_head(c, h):
                    cs = slice(CHUNK * c, CHUNK * (c + 1))
                    qm, qr = Q_POS[h]
                    km, kr = K_POS[h]
                    nat = nats[c]
                    qsl = QK[qr:qr + 64, qm, cs]       # [64, 256]
                    a_ps = aps.tile([128, 512], f32, tag="a")
                    for jb in range(2):
                        psl = slice(CHUNK * c + 128 * jb,
                                    CHUNK * c + 128 * jb + 128)
                        nc.tensor.matmul(
                            a_ps[:, 256 * jb:256 * jb + 256],
                            lhsT=QK[kr:kr + 64, km, psl],
                            rhs=qsl, start=True, stop=True,
                        )
                    a_sw = asb.tile([128, 512], bf16, tag="asw")
                    nc.vector.tensor_tensor(
                        a_sw[:], a_ps[:], mask_sb[:], op=Alu.mult,
                    )

                    oi = (c * HPC + h) % 2
                    o_ps = o_all[:, 256 * oi:256 * oi + 256]
                    first = True
                    if c > 0:
                        nc.tensor.matmul(
                            o_ps[:], lhsT=S_prev[h], rhs=qsl,
                            start=True, stop=False,
                        )
                        first = False
                    for jb in range(2):
                        nc.tensor.matmul(
                            o_ps[:],
                            lhsT=nat[:, jb, V_COL[h]:V_COL[h] + 128],
                            rhs=a_sw[:, 256 * jb:256 * jb + 256],
                            start=first, stop=(jb == 1),
                        )
                        first = False

                    s_ps = s_all[:, oi]
                    for jb in range(2):
                        nc.tensor.matmul(
                            s_ps,
                            lhsT=nat[:, jb, K_COL[h]:K_COL[h] + 64],
                            rhs=nat[:, jb, V_COL[h]:V_COL[h] + 128],
                            start=(jb == 0), stop=(jb == 1),
                        )
                    if c < NCH - 1:
                        s_big = ssb.tile([128, 128], bf16, tag="ssb")
                        s_new = s_big[kr:kr + 64, :]
                        if c == 0:
                            nc.vector.tensor_copy(out=s_new, in_=s_ps)
                        else:
                            nc.vector.tensor_tensor(
                                s_new, s_ps, S_prev[h], op=Alu.add
                            )
                        S_prev[h] = s_new

                    rec = rcb.tile([64, 256], f32, tag="rec")
                    nc.vector.reciprocal(rec[:], o_ps[64:128, :])
                    if h == 0:
                        nc.vector.tensor_tensor(
                            attnT01[0:64, cs], o_ps[0:64, :], rec[:],
                            op=Alu.mult,
                        )
                    elif h == 1:
                        nc.vector.tensor_tensor(
                            attnT01[64:128, cs], o_ps[0:64, :], rec[:],
                            op=Alu.mult,
                        )
                    else:
                        nc.vector.tensor_tensor(
                            attnT2[:, cs], o_ps[0:64, :], rec[:],
                            op=Alu.mult,
                        )

                for lc in range(2):
                    sl = slice(512 * lc, 512 * lc + 512)

                    # gates first (sigmoid table batch)
                    z = mm6(0, sl)
                    gbias0 = gb_sb[:, 0:1] if has_gate_bias else 0.0
                    nc.scalar.activation(
                        out=gf[:, sl], in_=z[:], func=Act.Sigmoid, bias=gbias0
                    )
                    z = mm6(1, sl)
                    gbias2 = gb_sb[0:64, 1:2] if has_gate_bias else 0.0
                    nc.scalar.activation(
                        out=gf2[:, sl], in_=z[0:64], func=Act.Sigmoid,
                        bias=gbias2,
                    )
                    corr(nc.vector, z, 1, slice(64, 128), sl)   # v0

                    # k tiles
                    z = mm6(2, sl)
                    corr(nc.vector, z, 2, slice(0, 128), sl)    # k0|k1
                    nc.vector.tensor_tensor(
                        QK[:, 2, sl], QK[:, 2, sl], gf[:, sl], op=Alu.mult
                    )
                    tmin_k01 = tmn.tile([128, 512], bf16, tag=f"tk01_{lc}",
                                        name=f"tk01_{lc}")
                    nc.gpsimd.tensor_scalar_min(
                        out=tmin_k01[:], in0=QK[:, 2, sl], scalar1=0.0
                    )
                    z = mm6(3, sl)
                    corr(nc.vector, z, 3, slice(0, 64), sl)     # k2
                    corr(nc.vector, z, 3, slice(64, 128), sl)   # v2
                    nc.vector.tensor_tensor(
                        QK[0:64, 3, sl], QK[0:64, 3, sl], gf2[:, sl],
                        op=Alu.mult,
                    )
                    tmin_k2 = tmn.tile([128, 512], bf16, tag=f"tk2_{lc}",
                                       name=f"tk2_{lc}")
                    nc.gpsimd.tensor_scalar_min(
                        out=tmin_k2[0:64], in0=QK[0:64, 3, sl], scalar1=0.0
                    )
                    # q tiles
                    z = mm6(4, sl)
                    corr(nc.vector, z, 4, slice(0, 128), sl)    # q0|q1
                    tmin_q01 = tmn.tile([128, 512], bf16, tag=f"tq01_{lc}",
                                        name=f"tq01_{lc}")
                    nc.gpsimd.tensor_scalar_min(
                        out=tmin_q01[:], in0=QK[:, 4, sl], scalar1=0.0
                    )
                    z = mm6(5, sl)
                    corr(nc.vector, z, 5, slice(0, 64), sl)     # q2
                    corr(nc.vector, z, 5, slice(64, 128), sl)   # v1
                    tmin_q2 = tmn.tile([128, 512], bf16, tag=f"tq2_{lc}",
                                       name=f"tq2_{lc}")
                    nc.gpsimd.tensor_scalar_min(
                        out=tmin_q2[0:64], in0=QK[0:64, 5, sl], scalar1=0.0
                    )

                    # deferred elu finalize: exp (scalar, one table batch)
                    # then QK = max(QK,0) + exp(min(QK,0))
                    for (m, rows, tm) in ((2, slice(0, 128), tmin_k01),
                                          (3, slice(0, 64), tmin_k2),
                                          (4, slice(0, 128), tmin_q01),
                                          (5, slice(0, 64), tmin_q2)):
                        texp = wk.tile([128, 512], bf16, tag="texp")
                        nc.scalar.activation(
                            out=texp[rows], in_=tm[rows], func=Act.Exp
                        )
                        nc.vector.scalar_tensor_tensor(
                            QK[rows, m, sl], in0=QK[rows, m, sl], scalar=0.0,
                            in1=texp[rows], op0=Alu.max, op1=Alu.add,
                        )

                    # PE transposes + nat copies for this half's 4 blocks
                    # (t_all slots: block parity picks slot group 0..3 / 4..7)
                    for bi in range(4):
                        c, jb = 2 * lc + bi // 2, bi % 2
                        psl = slice(512 * lc + 128 * bi, 512 * lc + 128 * bi + 128)
                        nat = nats[c]
                        sg = 4 * (bi % 2)
                        # t2 -> [k0n|k1n]
                        tp = t_all[:, sg + 0]
                        nc.tensor.transpose(tp, QK[:, 2, psl], id_sb[:])
                        nc.vector.tensor_copy(
                            out=nat[:, jb, K_COL[0]:K_COL[0] + 128], in_=tp
                        )
                        # t3 -> [k2n|v2n]
                        tp = t_all[:, sg + 1]
                        nc.tensor.transpose(tp, QK[:, 3, psl], id_sb[:])
                        nc.scalar.activation(
                            out=nat[:, jb, K_COL[2]:K_COL[2] + 64],
                            in_=tp[:, 0:64], func=Act.Copy,
                        )
                        nc.scalar.activation(
                            out=nat[:, jb, V_COL[2]:V_COL[2] + 64],
                            in_=tp[:, 64:128], func=Act.Copy,
                        )
                        # t1 -> v0n (upper half)
                        tp = t_all[:, sg + 2]
                        nc.tensor.transpose(tp, QK[:, 1, psl], id_sb[:])
                        nc.vector.tensor_copy(
                            out=nat[:, jb, V_COL[0]:V_COL[0] + 64],
                            in_=tp[:, 64:128],
                        )
                        # t5 -> v1n (upper half)
                        tp = t_all[:, sg + 3]
                        nc.tensor.transpose(tp, QK[:, 5, psl], id_sb[:])
                        nc.scalar.activation(
                            out=nat[:, jb, V_COL[1]:V_COL[1] + 64],
                            in_=tp[:, 64:128], func=Act.Copy,
                        )

                    # attention for this half's chunks
                    for c in (2 * lc, 2 * lc + 1):
                        for h in range(HPC):
                            attn_head(c, h)

                    # output projection for this half (feature-major);
                    # shares the a_ps pool's banks (attention for this half
                    # has finished issuing by now)
                    for f in range(6):
                        p_ps = aps.tile([128, 512], f32, tag="a")
                        nc.tensor.matmul(
                            p_ps[:], lhsT=wp01_sb[:, 128 * f:128 * f + 128],
                            rhs=attnT01[:, sl], start=True, stop=False,
                        )
                        nc.tensor.matmul(
                            p_ps[:], lhsT=wp2_sb[:, 128 * f:128 * f + 128],
                            rhs=attnT2[:, sl], start=False, stop=True,
                        )
                        po = posb.tile([128, 512], bf16, tag="po")
                        nc.scalar.activation(
                            out=po[:], in_=p_ps[:], func=Act.Copy
                        )
                        eng = nc.sync if f % 2 else nc.gpsimd
                        eng.dma_start(outPr[:, f, sl], po[:])

                    # gate out for this half
                    nc.sync.dma_start(gateT[0:128, sl], gf[:, sl])
                    nc.sync.dma_start(gateT[128:192, sl], gf2[:, sl])
    return nc


def _get_compiled(has_qkv_bias, has_gate_bias):
    key = (has_qkv_bias, has_gate_bias)
    if key not in _compiled:
        _compiled[key] = _build_nc(has_qkv_bias, has_gate_bias)
    return _compiled[key]


def _host_prep(x, W_qkv, b_qkv, W_gate, b_gate, W_proj, b_proj, ln_g, ln_b):
    """Build the 8 per-core input maps."""
    import ml_dtypes

    fp8 = ml_dtypes.float8_e4m3fn
    x = np.ascontiguousarray(np.asarray(x, np.float32))
    W_qkv = np.asarray(W_qkv, np.float32)
    W_gate = np.asarray(W_gate, np.float32)
    W_proj = np.asarray(W_proj, np.float32)
    ln_g = np.asarray(ln_g, np.float32)
    ln_b = np.asarray(ln_b, np.float32)
    b_qkv = np.asarray(b_qkv, np.float32)
    b_gate = np.asarray(b_gate, np.float32)

    W_eff = W_qkv * ln_g[:, None]
    # bias row folded through the LN affine: ln_b @ W_qkv + b_qkv
    qkv_bias_row = ln_b @ W_qkv + b_qkv

    p = np.arange(128)[:, None]
    i = np.arange(256)[None, :]
    mask = np.concatenate(
        [(p <= i).astype(np.float32), (p + 128 <= i).astype(np.float32)],
        axis=1,
    ).astype(ml_dtypes.bfloat16)
    ones8 = np.ones((128, 256), fp8)
    idI = np.eye(128, dtype=ml_dtypes.bfloat16)

    # per-batch fp8 x and x^2 (clip to TRN e4m3 range)
    x8 = [np.clip(x[b].T, -240, 240).astype(fp8) for b in range(B)]
    x28 = [np.clip(x8[b].astype(np.float32) ** 2, 0, 240).astype(fp8)
           for b in range(B)]

    in_maps = []
    for c in range(NCORES):
        b = c // GROUPS
        g = c % GROUPS
        hs = slice(192 * g, 192 * g + 192)
        Wq = W_eff[:, 0:768][:, hs]
        Wk = W_eff[:, 768:1536][:, hs]
        Wv = W_eff[:, 1536:2304][:, hs]
        Wg = W_gate[:, hs]
        bq = qkv_bias_row[0:768][hs]
        bk = qkv_bias_row[768:1536][hs]
        bv = qkv_bias_row[1536:2304][hs]
        bg = b_gate[hs]

        # t0=[g0|g1] t1=[g2|v0] t2=[k0|k1] t3=[k2|v2] t4=[q0|q1] t5=[q2|v1]
        tiles = [
            Wg[:, 0:128],
            np.concatenate([Wg[:, 128:192], Wv[:, 0:64]], axis=1),
            Wk[:, 0:128],
            np.concatenate([Wk[:, 128:192], Wv[:, 128:192]], axis=1),
            Wq[:, 0:128],
            np.concatenate([Wq[:, 128:192], Wv[:, 64:128]], axis=1),
        ]
        wAll = np.concatenate(tiles, axis=1)  # (768, 768)

        # negated column sums (LN correction), zero for gate columns
        negcs_a = np.zeros((6, 128), np.float32)
        qkvb = np.zeros((6, 128), np.float32)
        gateb = np.zeros((6, 128), np.float32)
        cs_q = Wq.sum(0); cs_k = Wk.sum(0); cs_v = Wv.sum(0)
        for h in range(HPC):
            mq, rq = Q_POS[h]; negcs_a[mq, rq:rq + 64] = -cs_q[64 * h:64 * h + 64]
            mk, rk = K_POS[h]; negcs_a[mk, rk:rk + 64] = -cs_k[64 * h:64 * h + 64]
            mv, rv = V_POS[h]; negcs_a[mv, rv:rv + 64] = -cs_v[64 * h:64 * h + 64]
            qkvb[mq, rq:rq + 64] = bq[64 * h:64 * h + 64]
            qkvb[mk, rk:rk + 64] = bk[64 * h:64 * h + 64]
            qkvb[mv, rv:rv + 64] = bv[64 * h:64 * h + 64]
            mg, rg = G_POS[h]; gateb[mg, rg:rg + 64] = bg[64 * h:64 * h + 64]

        in_maps.append({
            "xT": np.ascontiguousarray(x[b].T).astype(ml_dtypes.bfloat16),
            "x8T": x8[b],
            "x28T": x28[b],
            "wAll": np.ascontiguousarray(wAll).astype(ml_dtypes.bfloat16),
            "ones8": ones8,
            "idI": idI,
            "maskI": mask,
            "negcs": negcs_a,
            "wp01": np.ascontiguousarray(W_proj[hs, :][0:128]).astype(
                ml_dtypes.bfloat16),
            "wp2": np.ascontiguousarray(W_proj[hs, :][128:192]).astype(
                ml_dtypes.bfloat16),
            "_qkvb": qkvb,
            "_gateb": gateb,
        })
    return in_maps


def _finalize_in_maps(in_maps):
    has_qkv_bias = any(np.any(m["_qkvb"]) for m in in_maps)
    has_gate_bias = any(np.any(m["_gateb"]) for m in in_maps)
    for m in in_maps:
        qb = m.pop("_qkvb")
        gb = m.pop("_gateb")
        if has_qkv_bias:
            m["qkvbI"] = qb
        if has_gate_bias:
            m["gbI"] = gb
    return has_qkv_bias, has_gate_bias


def _assemble(results, b_proj):
    b_proj = np.asarray(b_proj, np.float32)
    out = np.zeros((B, L, D), np.float32)
    gate = np.zeros((B, L, D), np.float32)
    for c in range(NCORES):
        b = c // GROUPS
        g = c % GROUPS
        r = results[c]
        out[b] += r["outPT"].astype(np.float32).T
        gate[b][:, 192 * g:192 * g + 192] = r["gateT"].astype(np.float32).T
    out += b_proj
    return out, gate


def kernel(x, W_qkv, b_qkv, W_gate, b_gate, W_proj, b_proj, ln_g, ln_b):
    import concourse.bass_utils as bass_utils

    in_maps = _host_prep(x, W_qkv, b_qkv, W_gate, b_gate, W_proj, b_proj,
                         ln_g, ln_b)
    has_qkv_bias, has_gate_bias = _finalize_in_maps(in_maps)
    nc = _get_compiled(has_qkv_bias, has_gate_bias)
    res = bass_utils.run_bass_kernel_spmd(
        nc, in_maps, core_ids=list(range(NCORES))
    )
    return _assemble(res.results, b_proj)


# revision 15
# speedup vs baseline: 1.4885x; 1.4885x over previous
# Causal gated D2 (linear) attention — Trainium2 Bass kernel, 8 NeuronCores.
#
# Sharding: core c -> batch b = c // 4, head group g = c % 4 (heads 3g..3g+2).
# Each core computes LN-stats, qkv/gate projections, chunked causal linear
# attention for its 3 heads, and a partial output projection (its 192
# attention dims x full D). Host sums the 4 partial proj outputs per batch.
#
# v2 design notes (vs the 110us baseline):
#  - LN-stats matmuls run in fp8e4 DoubleRow (2x contraction, 0.5 cyc/row):
#    host uploads x and x^2 pre-quantized to e4m3. Projections stay bf16
#    (fp8 projections measured 3.5e-2 rel err -- too lossy).
#  - All DMA transposes replaced by PE-array transposes (matmul
#    is_transpose=True with an identity rhs) + PSUM->SBUF copies.
#  - Activation-table discipline: per lc-half the scalar engine runs
#    [sigmoid]*2 then [exp]*4 batches; stats use the shared ln+exp table.
#    Phase-2 reciprocals moved to the vector engine (reciprocal_approx_fast),
#    so phase 2 loads no tables at all.
#  - Output projection computed feature-major (out = W_proj^T @ attn^T,
#    moving dim 512), bf16 partials; host accumulates in f32.
#  - Emission order interleaves: stats -> [z(lc) -> elu(lc) -> transposes ->
#    attn chunks of lc -> proj-out(lc)] for lc in 0,1 so TensorMatrix stays
#    continuously busy while Vector/Scalar trail one stage behind.
#
# Feature packing (6 tiles of 128 partitions; per-head partition bases must
# match between q/k (matmul lhsT/rhs) and k/gate (elementwise)):
#   t0 = [g0 | g1]   t1 = [g2 | v0]   t2 = [k0 | k1]
#   t3 = [k2 | v2]   t4 = [q0 | q1]   t5 = [q2 | v1]
# Per 128-token block the PE transposes t2 ([k0n|k1n]), t3 ([k2n|v2n]),
# t1 (v0n half), t5 (v1n half) feed a nat tile
#   [v0n | 1 | v1n | 1 | v2n | 1 | k0n k1n k2n]  (cols 0..576)
# whose [v_h | ones] pairs are contiguous 128-col matmul operands.

import numpy as np

B, L, D, H = 2, 1024, 768, 12
DH = 64
NCORES = 8
HPC = 3          # heads per core
GROUPS = 4       # head groups per batch
CHUNK = 256      # attention chunk length
NCH = L // CHUNK

_compiled = {}


def _split_drain_tile_context():
    """TileContext that caps sem waits per instruction (this walrus build
    rejects instructions carrying more than one sync wait on several
    instruction templates) by moving excess waits onto preceding
    same-engine nops, and splits the tail drain the same way."""
    import concourse.tile as tile
    import concourse.mybir as mybir
    from concourse.tile import ScopedClock
    import bass_rust

    MAXW = 1

    class SplitDrainTileContext(tile.TileContext):
        _wsplit_counter = 0

        def _lower_ordered_insts(self, ordered):
            for bb_name, insts in ordered.items():
                need = any(
                    getattr(i, "sync_info", None) is not None
                    and len(i.sync_info.on_wait) > MAXW
                    for i in insts
                )
                if not need:
                    continue
                new_list = []
                for inst in insts:
                    si = getattr(inst, "sync_info", None)
                    if si is not None and len(si.on_wait) > MAXW:
                        waits = list(si.on_wait)
                        si.on_wait.clear()
                        for w in waits[MAXW:]:
                            SplitDrainTileContext._wsplit_counter += 1
                            nop = mybir.InstNoOp(
                                name=f"__wsplit_{SplitDrainTileContext._wsplit_counter}",
                                ins=[], outs=[],
                            )
                            nop.engine = inst.engine
                            nop.sync_info = bass_rust.SyncInfo(
                                on_wait=[w], on_update=[]
                            )
                            new_list.append(nop)
                        for w in waits[:MAXW]:
                            si.on_wait.append(w)
                    new_list.append(inst)
                insts[:] = new_list
            return super()._lower_ordered_insts(ordered)

        def _drain_and_barrier(self, tick_clock, wait_clock):
            drain_inst = self.nc.sync.drain()
            wait_clock.add_sem_waits(
                drain_inst.ins, ScopedClock({None: tick_clock.global_clock})
            )
            si = drain_inst.ins.sync_info
            if si is not None and len(si.on_wait) > 1:
                waits = list(si.on_wait)
                si.on_wait.clear()
                si.on_wait.append(waits[0])
                for w in waits[1:]:
                    d2 = self.nc.sync.drain()
                    si2 = d2.ins.sync_info
                    if si2 is None:
                        d2.ins.sync_info = bass_rust.SyncInfo(
                            on_wait=[w], on_update=[]
                        )
                    else:
                        si2.on_wait.append(w)
            self.nc.all_engine_barrier()
            assert self.sems is not None
            popped = self.nc._tile_sem_poison_stack.pop()
            assert popped is self._sem_poison
            self.nc.clear_and_free_semaphores(list(self.sems.allocated().values()))
            self.nc.all_engine_barrier()

    return SplitDrainTileContext


# (tile, partition base) per head; q/k/g share a base per head.
G_POS = [(0, 0), (0, 64), (1, 0)]
V_POS = [(1, 64), (5, 64), (3, 64)]
K_POS = [(2, 0), (2, 64), (3, 0)]
Q_POS = [(4, 0), (4, 64), (5, 0)]

# nat tile columns
V_COL = [0, 128, 256]          # start of [v_h | ones] 128-col run
K_COL = [384, 448, 512]        # start of k_h nat 64-col run


def _build_nc(has_qkv_bias, has_gate_bias):
    import concourse.bass as bass
    import concourse.mybir as mybir

    f32 = mybir.dt.float32
    bf16 = mybir.dt.bfloat16
    fp8 = mybir.dt.float8e4
    Alu = mybir.AluOpType
    Act = mybir.ActivationFunctionType
    DR = mybir.MatmulPerfMode.DoubleRow

    TC = _split_drain_tile_context()

    nc = bass.Bass()
    # ---- DRAM I/O ----
    xT = nc.dram_tensor("xT", (D, L), bf16, kind="ExternalInput")
    x8T = nc.dram_tensor("x8T", (D, L), fp8, kind="ExternalInput")
    x28T = nc.dram_tensor("x28T", (D, L), fp8, kind="ExternalInput")
    wAll = nc.dram_tensor("wAll", (D, 768), bf16, kind="ExternalInput")
    ones8 = nc.dram_tensor("ones8", (128, 256), fp8, kind="ExternalInput")
    idI = nc.dram_tensor("idI", (128, 128), bf16, kind="ExternalInput")
    maskI = nc.dram_tensor("maskI", (128, 512), bf16, kind="ExternalInput")
    negcs = nc.dram_tensor("negcs", (6, 128), f32, kind="ExternalInput")
    wp01 = nc.dram_tensor("wp01", (128, D), bf16, kind="ExternalInput")
    wp2 = nc.dram_tensor("wp2", (64, D), bf16, kind="ExternalInput")
    if has_qkv_bias:
        qkvbI = nc.dram_tensor("qkvbI", (6, 128), f32, kind="ExternalInput")
    if has_gate_bias:
        gbI = nc.dram_tensor("gbI", (6, 128), f32, kind="ExternalInput")
    outPT = nc.dram_tensor("outPT", (D, L), bf16, kind="ExternalOutput")
    gateT = nc.dram_tensor("gateT", (192, L), bf16, kind="ExternalOutput")

    with TC(nc) as tc:
        with tc.tile_pool(name="const", bufs=1) as cp:
            # ---- persistent SBUF ----
            x8_sb = cp.tile([128, 6, L], fp8)
            x28_sb = cp.tile([128, 6, L], fp8)
            xT_sb = cp.tile([128, 6, L], bf16)
            wAll_sb = cp.tile([128, 6, 768], bf16)
            ones8_sb = cp.tile([128, 2, 128], fp8)
            id_sb = cp.tile([128, 128], bf16)
            mask_sb = cp.tile([128, 512], bf16)
            negcs_sb = cp.tile([128, 6], f32)
            wp01_sb = cp.tile([128, 768], bf16)
            wp2_sb = cp.tile([64, 768], bf16)

            x8r = x8T.rearrange("(ks p) l -> p ks l", p=128)
            x28r = x28T.rearrange("(ks p) l -> p ks l", p=128)
            xTr = xT.rearrange("(ks p) l -> p ks l", p=128)
            wAr = wAll.rearrange("(ks p) m -> p ks m", p=128)
            outPr = outPT.rearrange("(ks p) l -> p ks l", p=128)

            # input DMAs, spread across queues, earliest-needed first
            nc.sync.dma_start(x8_sb[:, :, 0:512], x8r[:, :, 0:512])
            nc.sync.dma_start(x28_sb[:, :, 0:512], x28r[:, :, 0:512])
            nc.scalar.dma_start(x8_sb[:, :, 512:1024], x8r[:, :, 512:1024])
            nc.scalar.dma_start(x28_sb[:, :, 512:1024], x28r[:, :, 512:1024])
            nc.scalar.dma_start(wAll_sb[:, :, 0:384], wAr[:, :, 0:384])
            nc.scalar.dma_start(wAll_sb[:, :, 384:768], wAr[:, :, 384:768])
            nc.gpsimd.dma_start(xT_sb[:, :, 0:512], xTr[:, :, 0:512])
            nc.gpsimd.dma_start(xT_sb[:, :, 512:1024], xTr[:, :, 512:1024])
            nc.gpsimd.dma_start(wp01_sb[:], wp01[:])
            nc.gpsimd.dma_start(wp2_sb[:], wp2[:])
            nc.sync.dma_start(ones8_sb[:], ones8.rearrange("p (t m) -> p t m", t=2))
            nc.sync.dma_start(id_sb[:], idI[:])
            nc.sync.dma_start(mask_sb[:], maskI[:])
            nc.sync.dma_start(negcs_sb[:], negcs.rearrange("m p -> p m"))
            if has_qkv_bias:
                qkvb_sb = cp.tile([128, 6], f32)
                nc.sync.dma_start(qkvb_sb[:], qkvbI.rearrange("m p -> p m"))
            if has_gate_bias:
                gb_sb = cp.tile([128, 6], f32)
                nc.sync.dma_start(gb_sb[:], gbI.rearrange("m p -> p m"))

            epsb = cp.tile([128, 1], f32)
            nc.vector.memset(epsb[:], 1e-5)
            eps6 = cp.tile([64, 1], f32)
            nc.vector.memset(eps6[:], 1e-6)

            QK = cp.tile([128, 6, L], bf16)       # corrected q/k/v store
            gf = cp.tile([128, L], bf16)          # gate [g0|g1]
            gf2 = cp.tile([64, L], bf16)          # gate g2
            rstdB = cp.tile([128, L], f32)
            muB = cp.tile([128, L], f32)
            attnT01 = cp.tile([128, L], bf16)     # attention out heads 0,1
            attnT2 = cp.tile([64, L], bf16)       # attention out head 2

            # nat tiles, one per chunk: [128 tok, 2 jb, 576]
            nats = [cp.tile([128, 2, 576], bf16, name=f"nat{c}")
                    for c in range(NCH)]
            for c in range(NCH):
                for h in range(HPC):
                    eng = nc.vector if (c + h) % 2 else nc.gpsimd
                    eng.memset(nats[c][:, :, V_COL[h] + 64:V_COL[h] + 128], 1.0)

            # ---------- LN stats (fp8 DoubleRow) ----------
            with tc.tile_pool(name="stps", bufs=4, space="PSUM") as stps, \
                 tc.tile_pool(name="stwk", bufs=4) as stwk:
                st_ps = []
                for lc in range(2):
                    sl = slice(512 * lc, 512 * lc + 512)
                    sum_ps = stps.tile([128, 512], f32, tag="st",
                                       name=f"sum{lc}")
                    ssq_ps = stps.tile([128, 512], f32, tag="st",
                                       name=f"ssq{lc}")
                    for t in range(3):
                        nc.tensor.matmul(
                            sum_ps[:], lhsT=ones8_sb[:],
                            rhs=x8_sb[:, 2 * t:2 * t + 2, sl],
                            start=(t == 0), stop=(t == 2), perf_mode=DR,
                        )
                    for t in range(3):
                        nc.tensor.matmul(
                            ssq_ps[:], lhsT=ones8_sb[:],
                            rhs=x28_sb[:, 2 * t:2 * t + 2, sl],
                            start=(t == 0), stop=(t == 2), perf_mode=DR,
                        )
                    st_ps.append((sum_ps, ssq_ps))
                for lc in range(2):
                    sl = slice(512 * lc, 512 * lc + 512)
                    sum_ps, ssq_ps = st_ps[lc]
                    nc.scalar.activation(
                        out=muB[:, sl], in_=sum_ps[:], func=Act.Copy,
                        scale=1.0 / D,
                    )
                    msq = stwk.tile([128, 512], f32, tag="msq")
                    nc.scalar.activation(
                        out=msq[:], in_=muB[:, sl], func=Act.Square,
                    )
                    var = stwk.tile([128, 512], f32, tag="var")
                    nc.vector.scalar_tensor_tensor(
                        var[:], in0=ssq_ps[:], scalar=1.0 / D, in1=msq[:],
                        op0=Alu.mult, op1=Alu.subtract,
                    )
                    # rstd = exp(-0.5 * ln(var + eps))
                    nc.scalar.activation(
                        out=var[:], in_=var[:], func=Act.Ln, bias=epsb[:, 0:1]
                    )
                    nc.scalar.activation(
                        out=rstdB[:, sl], in_=var[:], func=Act.Exp, scale=-0.5
                    )

            # ---------- projections + attention, interleaved per lc ----------
            with tc.tile_pool(name="zps", bufs=3, space="PSUM") as zps, \
                 tc.tile_pool(name="tps", bufs=1, space="PSUM") as tps, \
                 tc.tile_pool(name="aps", bufs=2, space="PSUM") as aps, \
                 tc.tile_pool(name="ops", bufs=1, space="PSUM") as ops, \
                 tc.tile_pool(name="sps", bufs=1, space="PSUM") as sps, \
                 tc.tile_pool(name="wk", bufs=3) as wk, \
                 tc.tile_pool(name="tmn", bufs=8) as tmn, \
                 tc.tile_pool(name="asb", bufs=3) as asb, \
                 tc.tile_pool(name="rcb", bufs=3) as rcb, \
                 tc.tile_pool(name="ssb", bufs=6) as ssb, \
                 tc.tile_pool(name="posb", bufs=2) as posb:

                # persistent PSUM tiles with manual region alternation
                # (per-buffer PSUM allocation is bank-granular, so pools of
                # small tiles waste banks; regions of one tile still get
                # region-level dependency tracking)
                t_all = tps.tile([128, 8, 128], bf16, name="t_all")  # 1 bank
                o_all = ops.tile([128, 512], f32, name="o_all")      # 1 bank
                s_all = sps.tile([64, 2, 128], f32, name="s_all")    # 1 bank

                def mm6(m, sl):
                    z = zps.tile([128, 512], f32, tag="z")
                    for ks in range(6):
                        nc.tensor.matmul(
                            z[:], lhsT=wAll_sb[:, ks, 128 * m:128 * m + 128],
                            rhs=xT_sb[:, ks, sl],
                            start=(ks == 0), stop=(ks == 5),
                        )
                    return z

                def corr(eng, z, m, rows, sl):
                    # QK[rows, m, sl] = (z + mu*negcs) * rstd  (+ bias)
                    tq = wk.tile([128, 512], f32, tag="tq")
                    eng.scalar_tensor_tensor(
                        tq[rows], in0=muB[rows, sl],
                        scalar=negcs_sb[rows, m:m + 1],
                        in1=z[rows], op0=Alu.mult, op1=Alu.add,
                    )
                    if has_qkv_bias:
                        t2q = wk.tile([128, 512], f32, tag="t2q")
                        eng.tensor_tensor(
                            t2q[rows], tq[rows], rstdB[rows, sl], op=Alu.mult
                        )
                        eng.tensor_scalar_add(
                            out=QK[rows, m, sl], in0=t2q[rows],
                            scalar1=qkvb_sb[rows, m:m + 1],
                        )
                    else:
                        eng.tensor_tensor(
                            QK[rows, m, sl], tq[rows], rstdB[rows, sl],
                            op=Alu.mult,
                        )

                S_prev = [None] * HPC

                def attn_head(c, h):
                    cs = slice(CHUNK * c, CHUNK * (c + 1))
                    qm, qr = Q_POS[h]
                    km, kr = K_POS[h]
                    nat = nats[c]
                    qsl = QK[qr:qr + 64, qm, cs]       # [64, 256]
                    a_ps = aps.tile([128, 512], f32, tag="a")
                    for jb in range(2):
                        psl = slice(CHUNK * c + 128 * jb,
                                    CHUNK * c + 128 * jb + 128)
                        nc.tensor.matmul(
                            a_ps[:, 256 * jb:256 * jb + 256],
                            lhsT=QK[kr:kr + 64, km, psl],
                            rhs=qsl, start=True, stop=True,
                        )
                    a_sw = asb.tile([128, 512], bf16, tag="asw")
                    nc.vector.tensor_tensor(
                        a_sw[:], a_ps[:], mask_sb[:], op=Alu.mult,
                    )

                    oi = (c * HPC + h) % 2
                    o_ps = o_all[:, 256 * oi:256 * oi + 256]
                    first = True
                    if c > 0:
                        nc.tensor.matmul(
                            o_ps[:], lhsT=S_prev[h], rhs=qsl,
                            start=True, stop=False,
                        )
                        first = False
                    for jb in range(2):
                        nc.tensor.matmul(
                            o_ps[:],
                            lhsT=nat[:, jb, V_COL[h]:V_COL[h] + 128],
                            rhs=a_sw[:, 256 * jb:256 * jb + 256],
                            start=first, stop=(jb == 1),
                        )
                        first = False

                    s_ps = s_all[:, oi]
                    for jb in range(2):
                        nc.tensor.matmul(
                            s_ps,
                            lhsT=nat[:, jb, K_COL[h]:K_COL[h] + 64],
                            rhs=nat[:, jb, V_COL[h]:V_COL[h] + 128],
                            start=(jb == 0), stop=(jb == 1),
                        )
                    if c < NCH - 1:
                        s_big = ssb.tile([128, 128], bf16, tag="ssb")
                        s_new = s_big[kr:kr + 64, :]
                        if c == 0:
                            nc.vector.tensor_copy(out=s_new, in_=s_ps)
                        else:
                            nc.vector.tensor_tensor(
                                s_new, s_ps, S_prev[h], op=Alu.add
                            )
                        S_prev[h] = s_new

                    den = rcb.tile([64, 256], f32, tag="den")
                    nc.scalar.activation(
                        out=den[:], in_=o_ps[64:128, :], func=Act.Ln,
                        bias=eps6[:, 0:1],
                    )
                    rec = rcb.tile([64, 256], f32, tag="rec")
                    nc.scalar.activation(
                        out=rec[:], in_=den[:], func=Act.Exp, scale=-1.0,
                    )
                    if h == 0:
                        nc.vector.tensor_tensor(
                            attnT01[0:64, cs], o_ps[0:64, :], rec[:],
                            op=Alu.mult,
                        )
                    elif h == 1:
                        nc.vector.tensor_tensor(
                            attnT01[64:128, cs], o_ps[0:64, :], rec[:],
                            op=Alu.mult,
                        )
                    else:
                        nc.vector.tensor_tensor(
                            attnT2[:, cs], o_ps[0:64, :], rec[:],
                            op=Alu.mult,
                        )

                for lc in range(2):
                    sl = slice(512 * lc, 512 * lc + 512)

                    # gates first (sigmoid table batch)
                    z = mm6(0, sl)
                    gbias0 = gb_sb[:, 0:1] if has_gate_bias else 0.0
                    nc.scalar.activation(
                        out=gf[:, sl], in_=z[:], func=Act.Sigmoid, bias=gbias0
                    )
                    z = mm6(1, sl)
                    gbias2 = gb_sb[0:64, 1:2] if has_gate_bias else 0.0
                    nc.scalar.activation(
                        out=gf2[:, sl], in_=z[0:64], func=Act.Sigmoid,
                        bias=gbias2,
                    )
                    corr(nc.vector, z, 1, slice(64, 128), sl)   # v0

                    # k tiles
                    z = mm6(2, sl)
                    corr(nc.vector, z, 2, slice(0, 128), sl)    # k0|k1
                    nc.vector.tensor_tensor(
                        QK[:, 2, sl], QK[:, 2, sl], gf[:, sl], op=Alu.mult
                    )
                    tmin_k01 = tmn.tile([128, 512], bf16, tag=f"tk01_{lc}",
                                        name=f"tk01_{lc}")
                    nc.vector.tensor_scalar_min(
                        out=tmin_k01[:], in0=QK[:, 2, sl], scalar1=0.0
                    )
                    z = mm6(3, sl)
                    corr(nc.vector, z, 3, slice(0, 64), sl)     # k2
                    corr(nc.vector, z, 3, slice(64, 128), sl)   # v2
                    nc.vector.tensor_tensor(
                        QK[0:64, 3, sl], QK[0:64, 3, sl], gf2[:, sl],
                        op=Alu.mult,
                    )
                    tmin_k2 = tmn.tile([128, 512], bf16, tag=f"tk2_{lc}",
                                       name=f"tk2_{lc}")
                    nc.vector.tensor_scalar_min(
                        out=tmin_k2[0:64], in0=QK[0:64, 3, sl], scalar1=0.0
                    )
                    # q tiles
                    z = mm6(4, sl)
                    corr(nc.vector, z, 4, slice(0, 128), sl)    # q0|q1
                    tmin_q01 = tmn.tile([128, 512], bf16, tag=f"tq01_{lc}",
                                        name=f"tq01_{lc}")
                    nc.vector.tensor_scalar_min(
                        out=tmin_q01[:], in0=QK[:, 4, sl], scalar1=0.0
                    )
                    z = mm6(5, sl)
                    corr(nc.vector, z, 5, slice(0, 64), sl)     # q2
                    corr(nc.vector, z, 5, slice(64, 128), sl)   # v1
                    tmin_q2 = tmn.tile([128, 512], bf16, tag=f"tq2_{lc}",
                                       name=f"tq2_{lc}")
                    nc.vector.tensor_scalar_min(
                        out=tmin_q2[0:64], in0=QK[0:64, 5, sl], scalar1=0.0
                    )

                    # deferred elu finalize: exp (scalar, one table batch)
                    # then QK = max(QK,0) + exp(min(QK,0))
                    for (m, rows, tm) in ((2, slice(0, 128), tmin_k01),
                                          (3, slice(0, 64), tmin_k2),
                                          (4, slice(0, 128), tmin_q01),
                                          (5, slice(0, 64), tmin_q2)):
                        texp = wk.tile([128, 512], bf16, tag="texp")
                        nc.scalar.activation(
                            out=texp[rows], in_=tm[rows], func=Act.Exp
                        )
                        nc.vector.scalar_tensor_tensor(
                            QK[rows, m, sl], in0=QK[rows, m, sl], scalar=0.0,
                            in1=texp[rows], op0=Alu.max, op1=Alu.add,
                        )

                    # PE transposes + nat copies for this half's 4 blocks
                    # (t_all slots: block parity picks slot group 0..3 / 4..7)
                    for bi in range(4):
                        c, jb = 2 * lc + bi // 2, bi % 2
                        psl = slice(512 * lc + 128 * bi, 512 * lc + 128 * bi + 128)
                        nat = nats[c]
                        sg = 4 * (bi % 2)
                        # t2 -> [k0n|k1n]
                        tp = t_all[:, sg + 0]
                        nc.tensor.transpose(tp, QK[:, 2, psl], id_sb[:])
                        nc.vector.tensor_copy(
                            out=nat[:, jb, K_COL[0]:K_COL[0] + 128], in_=tp
                        )
                        # t3 -> [k2n|v2n]
                        tp = t_all[:, sg + 1]
                        nc.tensor.transpose(tp, QK[:, 3, psl], id_sb[:])
                        nc.scalar.activation(
                            out=nat[:, jb, K_COL[2]:K_COL[2] + 64],
                            in_=tp[:, 0:64], func=Act.Copy,
                        )
                        nc.scalar.activation(
                            out=nat[:, jb, V_COL[2]:V_COL[2] + 64],
                            in_=tp[:, 64:128], func=Act.Copy,
                        )
                        # t1 -> v0n (upper half)
                        tp = t_all[:, sg + 2]
                        nc.tensor.transpose(tp, QK[:, 1, psl], id_sb[:])
                        nc.vector.tensor_copy(
                            out=nat[:, jb, V_COL[0]:V_COL[0] + 64],
                            in_=tp[:, 64:128],
                        )
                        # t5 -> v1n (upper half)
                        tp = t_all[:, sg + 3]
                        nc.tensor.transpose(tp, QK[:, 5, psl], id_sb[:])
                        nc.scalar.activation(
                            out=nat[:, jb, V_COL[1]:V_COL[1] + 64],
                            in_=tp[:, 64:128], func=Act.Copy,
                        )

                    # attention for this half's chunks
                    for c in (2 * lc, 2 * lc + 1):
                        for h in range(HPC):
                            attn_head(c, h)

                    # output projection for this half (feature-major);
                    # shares the a_ps pool's banks (attention for this half
                    # has finished issuing by now)
                    for f in range(6):
                        p_ps = aps.tile([128, 512], f32, tag="a")
                        nc.tensor.matmul(
                            p_ps[:], lhsT=wp01_sb[:, 128 * f:128 * f + 128],
                            rhs=attnT01[:, sl], start=True, stop=False,
                        )
                        nc.tensor.matmul(
                            p_ps[:], lhsT=wp2_sb[:, 128 * f:128 * f + 128],
                            rhs=attnT2[:, sl], start=False, stop=True,
                        )
                        po = posb.tile([128, 512], bf16, tag="po")
                        nc.scalar.activation(
                            out=po[:], in_=p_ps[:], func=Act.Copy
                        )
                        eng = nc.sync if f % 2 else nc.gpsimd
                        eng.dma_start(outPr[:, f, sl], po[:])

                    # gate out for this half
                    nc.sync.dma_start(gateT[0:128, sl], gf[:, sl])
                    nc.sync.dma_start(gateT[128:192, sl], gf2[:, sl])
    return nc


def _get_compiled(has_qkv_bias, has_gate_bias):
    key = (has_qkv_bias, has_gate_bias)
    if key not in _compiled:
        _compiled[key] = _build_nc(has_qkv_bias, has_gate_bias)
    return _compiled[key]


def _host_prep(x, W_qkv, b_qkv, W_gate, b_gate, W_proj, b_proj, ln_g, ln_b):
    """Build the 8 per-core input maps."""
    import ml_dtypes

    fp8 = ml_dtypes.float8_e4m3fn
    x = np.ascontiguousarray(np.asarray(x, np.float32))
    W_qkv = np.asarray(W_qkv, np.float32)
    W_gate = np.asarray(W_gate, np.float32)
    W_proj = np.asarray(W_proj, np.float32)
    ln_g = np.asarray(ln_g, np.float32)
    ln_b = np.asarray(ln_b, np.float32)
    b_qkv = np.asarray(b_qkv, np.float32)
    b_gate = np.asarray(b_gate, np.float32)

    W_eff = W_qkv * ln_g[:, None]
    # bias row folded through the LN affine: ln_b @ W_qkv + b_qkv
    qkv_bias_row = ln_b @ W_qkv + b_qkv

    p = np.arange(128)[:, None]
    i = np.arange(256)[None, :]
    mask = np.concatenate(
        [(p <= i).astype(np.float32), (p + 128 <= i).astype(np.float32)],
        axis=1,
    ).astype(ml_dtypes.bfloat16)
    ones8 = np.ones((128, 256), fp8)
    idI = np.eye(128, dtype=ml_dtypes.bfloat16)

    # per-batch fp8 x and x^2 (clip to TRN e4m3 range)
    x8 = [np.clip(x[b].T, -240, 240).astype(fp8) for b in range(B)]
    x28 = [np.clip(x8[b].astype(np.float32) ** 2, 0, 240).astype(fp8)
           for b in range(B)]

    in_maps = []
    for c in range(NCORES):
        b = c // GROUPS
        g = c % GROUPS
        hs = slice(192 * g, 192 * g + 192)
        Wq = W_eff[:, 0:768][:, hs]
        Wk = W_eff[:, 768:1536][:, hs]
        Wv = W_eff[:, 1536:2304][:, hs]
        Wg = W_gate[:, hs]
        bq = qkv_bias_row[0:768][hs]
        bk = qkv_bias_row[768:1536][hs]
        bv = qkv_bias_row[1536:2304][hs]
        bg = b_gate[hs]

        # t0=[g0|g1] t1=[g2|v0] t2=[k0|k1] t3=[k2|v2] t4=[q0|q1] t5=[q2|v1]
        tiles = [
            Wg[:, 0:128],
            np.concatenate([Wg[:, 128:192], Wv[:, 0:64]], axis=1),
            Wk[:, 0:128],
            np.concatenate([Wk[:, 128:192], Wv[:, 128:192]], axis=1),
            Wq[:, 0:128],
            np.concatenate([Wq[:, 128:192], Wv[:, 64:128]], axis=1),
        ]
        wAll = np.concatenate(tiles, axis=1)  # (768, 768)

        # negated column sums (LN correction), zero for gate columns
        negcs_a = np.zeros((6, 128), np.float32)
        qkvb = np.zeros((6, 128), np.float32)
        gateb = np.zeros((6, 128), np.float32)
        cs_q = Wq.sum(0); cs_k = Wk.sum(0); cs_v = Wv.sum(0)
        for h in range(HPC):
            mq, rq = Q_POS[h]; negcs_a[mq, rq:rq + 64] = -cs_q[64 * h:64 * h + 64]
            mk, rk = K_POS[h]; negcs_a[mk, rk:rk + 64] = -cs_k[64 * h:64 * h + 64]
            mv, rv = V_POS[h]; negcs_a[mv, rv:rv + 64] = -cs_v[64 * h:64 * h + 64]
            qkvb[mq, rq:rq + 64] = bq[64 * h:64 * h + 64]
            qkvb[mk, rk:rk + 64] = bk[64 * h:64 * h + 64]
            qkvb[mv, rv:rv + 64] = bv[64 * h:64 * h + 64]
            mg, rg = G_POS[h]; gateb[mg, rg:rg + 64] = bg[64 * h:64 * h + 64]

        in_maps.append({
            "xT": np.ascontiguousarray(x[b].T).astype(ml_dtypes.bfloat16),
            "x8T": x8[b],
            "x28T": x28[b],
            "wAll": np.ascontiguousarray(wAll).astype(ml_dtypes.bfloat16),
            "ones8": ones8,
            "idI": idI,
            "maskI": mask,
            "negcs": negcs_a,
            "wp01": np.ascontiguousarray(W_proj[hs, :][0:128]).astype(
                ml_dtypes.bfloat16),
            "wp2": np.ascontiguousarray(W_proj[hs, :][128:192]).astype(
                ml_dtypes.bfloat16),
            "_qkvb": qkvb,
            "_gateb": gateb,
        })
    return in_maps


def _finalize_in_maps(in_maps):
    has_qkv_bias = any(np.any(m["_qkvb"]) for m in in_maps)
    has_gate_bias = any(np.any(m["_gateb"]) for m in in_maps)
    for m in in_maps:
        qb = m.pop("_qkvb")
        gb = m.pop("_gateb")
        if has_qkv_bias:
            m["qkvbI"] = qb
        if has_gate_bias:
            m["gbI"] = gb
    return has_qkv_bias, has_gate_bias


def _assemble(results, b_proj):
    b_proj = np.asarray(b_proj, np.float32)
    out = np.zeros((B, L, D), np.float32)
    gate = np.zeros((B, L, D), np.float32)
    for c in range(NCORES):
        b = c // GROUPS
        g = c % GROUPS
        r = results[c]
        out[b] += r["outPT"].astype(np.float32).T
        gate[b][:, 192 * g:192 * g + 192] = r["gateT"].astype(np.float32).T
    out += b_proj
    return out, gate


def kernel(x, W_qkv, b_qkv, W_gate, b_gate, W_proj, b_proj, ln_g, ln_b):
    import concourse.bass_utils as bass_utils

    in_maps = _host_prep(x, W_qkv, b_qkv, W_gate, b_gate, W_proj, b_proj,
                         ln_g, ln_b)
    has_qkv_bias, has_gate_bias = _finalize_in_maps(in_maps)
    nc = _get_compiled(has_qkv_bias, has_gate_bias)
    res = bass_utils.run_bass_kernel_spmd(
        nc, in_maps, core_ids=list(range(NCORES))
    )
    return _assemble(res.results, b_proj)
